# revision 6
# baseline (speedup 1.0000x reference)
"""GATv2 state encoder on 8 Trainium2 NeuronCores (Bass/Tile), fused v3.

v3 = v2 + hardware For_i loops over the 49 edge blocks (blocks 0..47 in
a For_i_unrolled(max_unroll=4) loop with dynamic-offset gathers/slices,
block 48 peeled statically to handle the partial pool mask): 2559
instructions vs 8664, which cuts bass build and NEFF packaging time.
The static python-loop path is kept for dbg builds and nonzero biases.

Single-launch design: nodes sharded 8 ways (6250/core, padded to 6272).
Each core: dense conv1 xl/xr tables from its LOCAL x shard only ->
on-device AllGather of the xl table -> edge phase conv1 (dma_gather of
src/dst rows + one-hot scatter matmul; one-hot slot matrices generated
on device from a tiny slot table via iota + is_equal) -> h1 kept
transposed in SBUF (bf16) -> dense conv2 -> AllGather xl2 -> edge phase
conv2 -> mean-pool partial [1,32] as the only output. Host: sum
partials / N, @ Wout.T + b_out.

Transfer-optimized (the axon tunnel runs at ~30MB/s, so input bytes
dominate the wall): x shard shipped bf16 (1.6MB), gather indices
shipped UNREPLICATED [16, ...] (400KB) and replicated to 128 partitions
on device, slot table as uint8 (100KB), weights bf16, pmask + identity
matrix generated on device. ~2.3MB/core total vs ~115MB/core in v1.
"""
import os
import numpy as np
import ml_dtypes

N = 50000
NC = 8
NSH = N // NC              # 6250
P = 128
NBLK = (NSH + P - 1) // P  # 49
PADN = NBLK * P            # 6272
GN = NC * PADN             # 50176
HALF = 4 * PADN            # 25088 (int16-safe split of gathered tables)
KA = 8
KB = 8
KCH = KA + KB
S1, S2, S3 = KA * P // 16, KB * P // 16, KCH * P // 16
SB = S1 + S2 + S3          # 256
CT1, CE1, H1 = 128, 128, 2
CT2, CE2, H2 = 32, 64, 1

_cache = {}


def preprocess(edge_index):
    src = np.concatenate([np.asarray(edge_index[0], np.int64),
                          np.arange(N, dtype=np.int64)])
    dst = np.concatenate([np.asarray(edge_index[1], np.int64),
                          np.arange(N, dtype=np.int64)])
    order = np.argsort(dst, kind='stable')
    src, dst = src[order], dst[order]
    # node id -> row in the AllGather'd table (per-core shards padded 6272)
    srow = (src // NSH) * PADN + (src % NSH)
    core_of = dst // NSH
    starts = np.searchsorted(core_of, np.arange(NC + 1))

    cores = []
    for c in range(NC):
        sl = slice(starts[c], starts[c + 1])
        s, d = srow[sl], dst[sl] - c * NSH
        b = d >> 7
        side = (s >= HALF).astype(np.int64)
        grp = b * 2 + side
        o2 = np.argsort(grp, kind='stable')  # dst-order kept within groups
        s, d, b, side, grp = s[o2], d[o2], b[o2], side[o2], grp[o2]
        cnt = np.bincount(grp, minlength=NBLK * 2)
        assert cnt.max() <= KA * P, f"block overflow: {cnt.max()}"
        gstart = np.concatenate([[0], np.cumsum(cnt)[:-1]])
        pos = np.arange(s.shape[0]) - gstart[grp]
        k = side * KA + (pos >> 7)
        e = pos & 127

        flat = b * (KCH * P) + k * P + e
        srcs = np.zeros(NBLK * KCH * P, np.int64)
        slot = np.full(NBLK * KCH * P, 255, np.int32)
        dstl = np.zeros(NBLK * KCH * P, np.int64)
        srcs[flat] = s
        slot[flat] = d & 127
        dstl[flat] = d
        srcs = srcs.reshape(NBLK, KCH, P)
        slot = slot.reshape(NBLK, KCH, P)
        dstl = dstl.reshape(NBLK, KCH, P)

        valid = slot < 128
        sa = np.where(valid[:, :KA], srcs[:, :KA], 0)
        sbb = np.where(valid[:, KA:], srcs[:, KA:] - HALF, 0)
        dr = np.where(valid, dstl, 0)

        def wrapv(v, KC):  # [NBLK, KC, P] -> [NBLK, 16, KC*P//16]
            return v.reshape(NBLK, KC * P // 16, 16).swapaxes(1, 2) \
                .astype(np.int16)

        idx_all = np.concatenate(
            [wrapv(sa, KA), wrapv(sbb, KB), wrapv(dr, KCH)], axis=2) \
            .transpose(1, 0, 2).reshape(16, NBLK * SB)
        idx_all = np.ascontiguousarray(idx_all)

        # slot value per (edge-in-chunk, block*chunk) as u8 [128, NBLK*KCH]
        slot8 = np.ascontiguousarray(
            slot.transpose(2, 0, 1).reshape(P, NBLK * KCH).astype(np.uint8))
        cores.append(dict(idx_all=idx_all, slot8=slot8))
    return cores


def build_fused(has_b1l, has_b1r, has_b1o, has_b2l, has_b2r, has_b2o,
                dbg=False, use_loop=False):
    import concourse.mybir as mybir
    import concourse.tile as tile
    import concourse.bacc as bacc

    nc = bacc.Bacc("TRN2", num_devices=NC)
    dt = mybir.dt
    f32, bf16, i16 = dt.float32, dt.bfloat16, dt.int16
    i32, u8 = dt.int32, dt.uint8
    AF = mybir.ActivationFunctionType
    OP = mybir.AluOpType
    GRP = [list(range(NC))]
    DB = 7
    NMAC = (NBLK + DB - 1) // DB

    d_xT = nc.dram_tensor("xT", [P, PADN], bf16, kind="ExternalInput")
    d_idx = nc.dram_tensor("idx", [16, NBLK * SB], i16, kind="ExternalInput")
    d_slot = nc.dram_tensor("slot", [P, NBLK * KCH], u8,
                            kind="ExternalInput")
    d_WA1 = nc.dram_tensor("WA1", [P, CT1], bf16, kind="ExternalInput")
    d_WB1 = nc.dram_tensor("WB1", [P, CT1], bf16, kind="ExternalInput")
    d_at1a = nc.dram_tensor("at1a", [P, CE1], bf16, kind="ExternalInput")
    d_at1b = nc.dram_tensor("at1b", [P, CE1], bf16, kind="ExternalInput")
    d_WA2 = nc.dram_tensor("WA2", [P, CT2], bf16, kind="ExternalInput")
    d_WB2 = nc.dram_tensor("WB2", [P, CT2], bf16, kind="ExternalInput")
    d_at2a = nc.dram_tensor("at2a", [P, CE2], bf16, kind="ExternalInput")
    d_at2b = nc.dram_tensor("at2b", [P, CE2], bf16, kind="ExternalInput")
    biases = {}
    for nm, flag, ct in (("b1l", has_b1l, CT1), ("b1r", has_b1r, CT1),
                         ("b1o", has_b1o, CT1), ("b2l", has_b2l, CT2),
                         ("b2r", has_b2r, CT2), ("b2o", has_b2o, CT2)):
        if flag:
            biases[nm] = nc.dram_tensor(nm, [P, ct], f32,
                                        kind="ExternalInput")

    d_pool = nc.dram_tensor("pool_out", [1, CT2], f32, kind="ExternalOutput")
    d_h1 = d_h2 = None
    if dbg:
        d_h1 = nc.dram_tensor("h1_dbg", [PADN, CT1], bf16,
                              kind="ExternalOutput")
        d_h2 = nc.dram_tensor("h2_dbg", [PADN, CT2], bf16,
                              kind="ExternalOutput")

    def edge_block(b, gat, gsm, epsum, t_idx, t_slot, t_iof,
                   tabA_ap, tabB_ap, tabR_ap, tdt, t_ata, t_atb,
                   CE, CT, H, t_bo, dyn=False):
        """One 128-dst-node block of a GATv2 edge phase. Returns t_h
        [P, CT] bf16 (relu'd node features for the block). With
        dyn=True, b is a For_i induction variable and all
        block-dependent offsets use dynamic slices."""
        import concourse.bass as bass

        def isl(off, size):  # idx-tile column slice
            return t_idx[:, bass.ds(off, size)] if dyn \
                else t_idx[:, off:off + size]

        CH = CT // H
        o = b * SB
        half3 = S3 // 2
        t_xl = gat.tile([P, KCH, CE], tdt, tag="xl")
        nc.gpsimd.dma_gather(
            out_ap=t_xl[:, 0:KA, :], in_ap=tabA_ap,
            idxs_ap=isl(o, S1),
            num_idxs=KA * P, num_idxs_reg=KA * P, elem_size=CE)
        nc.gpsimd.dma_gather(
            out_ap=t_xl[:, KA:KCH, :], in_ap=tabB_ap,
            idxs_ap=isl(o + S1, S2),
            num_idxs=KB * P, num_idxs_reg=KB * P, elem_size=CE)
        t_xr = gat.tile([P, KCH, CE], tdt, tag="xr")
        nc.gpsimd.dma_gather(
            out_ap=t_xr[:, 0:KCH // 2, :], in_ap=tabR_ap,
            idxs_ap=isl(o + S1 + S2, half3),
            num_idxs=KCH * P // 2, num_idxs_reg=KCH * P // 2, elem_size=CE)
        nc.gpsimd.dma_gather(
            out_ap=t_xr[:, KCH // 2:KCH, :], in_ap=tabR_ap,
            idxs_ap=isl(o + S1 + S2 + half3, half3),
            num_idxs=KCH * P // 2, num_idxs_reg=KCH * P // 2, elem_size=CE)

        # one-hot scatter matrices from slot values: msel[e,k,n] =
        # (slot[b,k,e] == n)
        t_msel = gsm.tile([P, KCH, P], bf16, tag="ms")
        if dyn:
            slot_sl = t_slot[:, bass.ds(b * KCH, KCH)]
        else:
            slot_sl = t_slot[:, b * KCH:(b + 1) * KCH]
        slot_b = slot_sl.unsqueeze(2).to_broadcast([P, KCH, P])
        iota_b = t_iof[:].unsqueeze(1).to_broadcast([P, KCH, P])
        nc.vector.tensor_tensor(out=t_msel[:], in0=slot_b, in1=iota_b,
                                op=OP.is_equal)

        t_z = gsm.tile([P, KCH, CE], bf16, tag="z")
        nc.vector.tensor_tensor(out=t_z[:], in0=t_xl[:], in1=t_xr[:],
                                op=OP.add)
        t_zp = gsm.tile([P, KCH, CE], bf16, tag="zp")
        nc.scalar.activation(t_zp[:], t_z[:], AF.Relu)
        # lrelu(z).att = (0.8 att).relu(z) + (0.2 att).z
        t_am = gsm.tile([P, KCH, 2, CE], bf16, tag="am")
        ata_b = t_ata[:].unsqueeze(1).to_broadcast([P, KCH, CE])
        nc.vector.tensor_tensor(out=t_am[:, :, 0, :], in0=t_zp[:],
                                in1=ata_b, op=OP.mult)
        atb_b = t_atb[:].unsqueeze(1).to_broadcast([P, KCH, CE])
        nc.vector.tensor_tensor(out=t_am[:, :, 1, :], in0=t_z[:],
                                in1=atb_b, op=OP.mult)
        t_red = gsm.tile([P, KCH, H], f32, tag="red")
        am_g = t_am[:].rearrange("p k s (h c) -> p k h s c", h=H)
        nc.vector.tensor_reduce(out=t_red[:], in_=am_g,
                                axis=mybir.AxisListType.XY, op=OP.add)
        t_ex = gsm.tile([P, KCH, H], f32, tag="ex")
        nc.scalar.activation(t_ex[:], t_red[:], AF.Exp)
        t_pay = gsm.tile([P, KCH, CE + H], bf16, tag="pay")
        CEH = CE // H
        ex_b = t_ex[:].unsqueeze(3).to_broadcast([P, KCH, H, CEH])
        pay4 = t_pay[:, :, 0:CE].rearrange("p k (h c) -> p k h c", h=H)
        xl4 = t_xl[:].rearrange("p k (h c) -> p k h c", h=H)
        nc.vector.tensor_tensor(out=pay4, in0=xl4, in1=ex_b, op=OP.mult)
        nc.vector.tensor_copy(t_pay[:, :, CE:CE + H], t_ex[:])

        t_seg = epsum.tile([P, CE + H], f32, tag="seg")
        for k in range(KCH):
            nc.tensor.matmul(t_seg[:], lhsT=t_msel[:, k, :],
                             rhs=t_pay[:, k, :],
                             start=(k == 0), stop=(k == KCH - 1))

        t_s = gsm.tile([P, H], f32, tag="s")
        nc.vector.tensor_scalar(out=t_s[:], in0=t_seg[:, CE:CE + H],
                                scalar1=1e-30, scalar2=None, op0=OP.max)
        t_rec = gsm.tile([P, H], f32, tag="rec")
        nc.vector.reciprocal(t_rec[:], t_s[:])
        t_hn = gsm.tile([P, CT], f32, tag="hn")
        rec_b = t_rec[:].unsqueeze(2).to_broadcast([P, H, CH])
        hn3 = t_hn[:].rearrange("p (h c) -> p h c", h=H)
        seg3 = t_seg[:, 0:CE].rearrange("p (h c) -> p h c", h=H)
        nc.vector.tensor_tensor(out=hn3, in0=seg3[:, :, 0:CH], in1=rec_b,
                                op=OP.mult)
        if t_bo is not None:
            t_hb = gsm.tile([P, CT], f32, tag="hb")
            nc.vector.tensor_tensor(out=t_hb[:], in0=t_hn[:], in1=t_bo[:],
                                    op=OP.add)
            t_hn = t_hb
        t_h = gsm.tile([P, CT], bf16, tag="h")
        nc.scalar.activation(t_h[:], t_hn[:], AF.Relu)
        return t_h

    with tile.TileContext(nc) as tc:
        with (
            tc.tile_pool(name="const", bufs=1) as constp,
            tc.tile_pool(name="dram", bufs=1, space="DRAM") as dram,
        ):
            # ---- persistent SBUF state ----
            t_idx = constp.tile([P, NBLK * SB], i16)
            for g in range(8):  # replicate the 16-partition wrap 8x
                nc.sync.dma_start(t_idx[g * 16:(g + 1) * 16, :], d_idx[:])
            t_slot8 = constp.tile([P, NBLK * KCH], u8)
            nc.sync.dma_start(t_slot8[:], d_slot[:])
            t_slot = constp.tile([P, NBLK * KCH], f32)
            nc.vector.tensor_copy(t_slot[:], t_slot8[:])
            t_WA1 = constp.tile([P, CT1], bf16)
            nc.sync.dma_start(t_WA1[:], d_WA1[:])
            t_WB1 = constp.tile([P, CT1], bf16)
            nc.sync.dma_start(t_WB1[:], d_WB1[:])
            t_at1a = constp.tile([P, CE1], bf16)
            nc.sync.dma_start(t_at1a[:], d_at1a[:])
            t_at1b = constp.tile([P, CE1], bf16)
            nc.sync.dma_start(t_at1b[:], d_at1b[:])
            t_WA2 = constp.tile([P, CT2], bf16)
            nc.sync.dma_start(t_WA2[:], d_WA2[:])
            t_WB2 = constp.tile([P, CT2], bf16)
            nc.sync.dma_start(t_WB2[:], d_WB2[:])
            t_at2a = constp.tile([P, CE2], bf16)
            nc.sync.dma_start(t_at2a[:], d_at2a[:])
            t_at2b = constp.tile([P, CE2], bf16)
            nc.sync.dma_start(t_at2b[:], d_at2b[:])
            t_bias = {}
            for nm, d_b in biases.items():
                ct = CT1 if nm.startswith("b1") else CT2
                t = constp.tile([P, ct], f32)
                nc.sync.dma_start(t[:], d_b[:])
                t_bias[nm] = t
            # iota 0..127 along free dim, same on every partition
            t_ioi = constp.tile([P, P], i32)
            nc.gpsimd.iota(t_ioi[:], pattern=[[1, P]], base=0,
                           channel_multiplier=0)
            t_iof = constp.tile([P, P], f32)
            nc.vector.tensor_copy(t_iof[:], t_ioi[:])
            # partition index [P, 1]
            t_pii = constp.tile([P, 1], i32)
            nc.gpsimd.iota(t_pii[:], pattern=[[0, 1]], base=0,
                           channel_multiplier=1)
            t_pif = constp.tile([P, 1], f32)
            nc.vector.tensor_copy(t_pif[:], t_pii[:])
            # identity matrix (bf16) for PE transposes
            t_id = constp.tile([P, P], bf16)
            nc.vector.tensor_scalar(out=t_id[:], in0=t_iof[:],
                                    scalar1=t_pif[:, 0:1], scalar2=None,
                                    op0=OP.is_equal)
            # pool mask [P, NBLK]: 1.0 where b*128+p < NSH
            t_pmi = constp.tile([P, NBLK], i32)
            nc.gpsimd.iota(t_pmi[:], pattern=[[P, NBLK]], base=0,
                           channel_multiplier=1)
            t_pmf = constp.tile([P, NBLK], f32)
            nc.vector.tensor_copy(t_pmf[:], t_pmi[:])
            t_pm = constp.tile([P, NBLK], bf16)
            nc.vector.tensor_scalar(out=t_pm[:], in0=t_pmf[:],
                                    scalar1=float(NSH), scalar2=None,
                                    op0=OP.is_lt)
            # h1 transposed [128ch, PADN nodes], filled by edge phase 1
            t_h1T = constp.tile([P, PADN], bf16)
            # x shard transposed, kept resident
            t_x = constp.tile([P, PADN], bf16)
            nc.sync.dma_start(t_x[:], d_xT[:])

            # ---- DRAM scratch ----
            dtL1 = dram.tile([PADN, CE1], bf16)
            dtR1 = dram.tile([PADN, CE1], bf16)
            dtL1g = dram.tile([GN, CE1], bf16)
            dtL2 = dram.tile([PADN, CE2], f32)
            dtR2 = dram.tile([PADN, CE2], f32)
            dtL2g = dram.tile([GN, CE2], f32)

            # ================= dense conv1 =================
            with (
                tc.tile_pool(name="d1out", bufs=3) as doutp,
                tc.tile_pool(name="d1ps", bufs=4, space="PSUM") as dpsum,
            ):
                for mc in range(NMAC):
                    c0 = mc * DB * P
                    cols = min(DB * P, PADN - c0)
                    nj = cols // P
                    t_a = doutp.tile([P, DB, CE1], bf16, tag="a")
                    t_b = doutp.tile([P, DB, CE1], bf16, tag="b")
                    for j in range(nj):
                        o0 = c0 + j * P
                        psA = dpsum.tile([P, CT1], f32, tag="psA")
                        nc.tensor.matmul(psA[:], lhsT=t_x[:, o0:o0 + P],
                                         rhs=t_WA1[:], start=True, stop=True)
                        if has_b1l:
                            nc.vector.tensor_tensor(
                                out=t_a[:, j, :], in0=psA[:],
                                in1=t_bias["b1l"][:], op=OP.add)
                        else:
                            nc.scalar.copy(t_a[:, j, :], psA[:])
                        psB = dpsum.tile([P, CT1], f32, tag="psB")
                        nc.tensor.matmul(psB[:], lhsT=t_x[:, o0:o0 + P],
                                         rhs=t_WB1[:], start=True, stop=True)
                        if has_b1r:
                            nc.vector.tensor_tensor(
                                out=t_b[:, j, :], in0=psB[:],
                                in1=t_bias["b1r"][:], op=OP.add)
                        else:
                            nc.scalar.copy(t_b[:, j, :], psB[:])
                    nc.sync.dma_start(
                        dtL1[c0:c0 + cols, :].rearrange("(j p) c -> p j c",
                                                        p=P),
                        t_a[:, 0:nj, :])
                    nc.sync.dma_start(
                        dtR1[c0:c0 + cols, :].rearrange("(j p) c -> p j c",
                                                        p=P),
                        t_b[:, 0:nj, :])

            nc.gpsimd.collective_compute(
                "AllGather", OP.bypass, replica_groups=GRP,
                ins=[dtL1[:].opt()], outs=[dtL1g[:].opt()])

            # ================= edge conv1 =================
            with (
                tc.tile_pool(name="e1g", bufs=2) as gat,
                tc.tile_pool(name="e1s", bufs=2) as gsm,
                tc.tile_pool(name="e1ps", bufs=2, space="PSUM") as epsum,
                tc.tile_pool(name="e1tp", bufs=2, space="PSUM") as tpsum,
            ):
                def c1_body(b, dyn):
                    import concourse.bass as bass
                    t_h = edge_block(
                        b, gat, gsm, epsum, t_idx, t_slot, t_iof,
                        dtL1g[0:HALF, :], dtL1g[HALF:GN, :], dtR1[:],
                        bf16, t_at1a, t_at1b, CE1, CT1, H1,
                        t_bias.get("b1o"), dyn=dyn)
                    ps_t = tpsum.tile([P, P], bf16, tag="tp")
                    nc.tensor.matmul(ps_t[:], lhsT=t_h[:], rhs=t_id[:],
                                     is_transpose=True)
                    if dyn:
                        nc.vector.tensor_copy(
                            t_h1T[:, bass.ds(b * P, P)], ps_t[:])
                    else:
                        nc.scalar.copy(t_h1T[:, b * P:(b + 1) * P],
                                       ps_t[:])
                        if dbg:
                            nc.sync.dma_start(d_h1[b * P:(b + 1) * P, :],
                                              t_h[:])
                    return t_h

                if use_loop:
                    tc.For_i_unrolled(0, NBLK - 1, 1,
                                      lambda iv: c1_body(iv, True),
                                      max_unroll=4)
                    c1_body(NBLK - 1, False)
                else:
                    for b in range(NBLK):
                        c1_body(b, False)

            # ================= dense conv2 =================
            with (
                tc.tile_pool(name="d2out", bufs=3) as doutp,
                tc.tile_pool(name="d2ps", bufs=4, space="PSUM") as dpsum,
            ):
                for mc in range(NMAC):
                    c0 = mc * DB * P
                    cols = min(DB * P, PADN - c0)
                    nj = cols // P
                    t_a = doutp.tile([P, DB, CE2], f32, tag="a")
                    t_b = doutp.tile([P, DB, CE2], f32, tag="b")
                    nc.vector.memset(t_a[:, :, CT2:CE2], 0.0)
                    nc.vector.memset(t_b[:, :, CT2:CE2], 0.0)
                    for j in range(nj):
                        o0 = c0 + j * P
                        psA = dpsum.tile([P, CT2], f32, tag="psA")
                        nc.tensor.matmul(psA[:], lhsT=t_h1T[:, o0:o0 + P],
                                         rhs=t_WA2[:], start=True, stop=True)
                        if has_b2l:
                            nc.vector.tensor_tensor(
                                out=t_a[:, j, 0:CT2], in0=psA[:],
                                in1=t_bias["b2l"][:], op=OP.add)
                        else:
                            nc.scalar.copy(t_a[:, j, 0:CT2], psA[:])
                        psB = dpsum.tile([P, CT2], f32, tag="psB")
                        nc.tensor.matmul(psB[:], lhsT=t_h1T[:, o0:o0 + P],
                                         rhs=t_WB2[:], start=True, stop=True)
                        if has_b2r:
                            nc.vector.tensor_tensor(
                                out=t_b[:, j, 0:CT2], in0=psB[:],
                                in1=t_bias["b2r"][:], op=OP.add)
                        else:
                            nc.scalar.copy(t_b[:, j, 0:CT2], psB[:])
                    nc.sync.dma_start(
                        dtL2[c0:c0 + cols, :].rearrange("(j p) c -> p j c",
                                                        p=P),
                        t_a[:, 0:nj, :])
                    nc.sync.dma_start(
                        dtR2[c0:c0 + cols, :].rearrange("(j p) c -> p j c",
                                                        p=P),
                        t_b[:, 0:nj, :])

            nc.gpsimd.collective_compute(
                "AllGather", OP.bypass, replica_groups=GRP,
                ins=[dtL2[:].opt()], outs=[dtL2g[:].opt()])

            # ================= edge conv2 + pool =================
            with (
                tc.tile_pool(name="e2g", bufs=2) as gat,
                tc.tile_pool(name="e2s", bufs=2) as gsm,
                tc.tile_pool(name="e2ps", bufs=2, space="PSUM") as epsum,
                tc.tile_pool(name="e2pp", bufs=1, space="PSUM") as ppsum,
            ):
                t_pool = ppsum.tile([1, CT2], f32)

                def c2_body(b, dyn, start, stop, pm_ap):
                    t_h = edge_block(
                        b, gat, gsm, epsum, t_idx, t_slot, t_iof,
                        dtL2g[0:HALF, :], dtL2g[HALF:GN, :], dtR2[:],
                        f32, t_at2a, t_at2b, CE2, CT2, H2,
                        t_bias.get("b2o"), dyn=dyn)
                    nc.tensor.matmul(t_pool[:], lhsT=pm_ap, rhs=t_h[:],
                                     start=start, stop=stop)
                    if dbg and not dyn:
                        nc.sync.dma_start(d_h2[b * P:(b + 1) * P, :],
                                          t_h[:])

                if use_loop:
                    # all-ones column works for full blocks; the only
                    # partially-valid block (last) is peeled with the mask
                    t_one1 = constp.tile([P, 1], bf16)
                    nc.vector.memset(t_one1[:], 1.0)
                    t_zero1 = constp.tile([P, 1], bf16)
                    nc.vector.memset(t_zero1[:], 0.0)
                    # zero-init the accumulating PSUM bank
                    nc.tensor.matmul(t_pool[:], lhsT=t_zero1[:],
                                     rhs=t_at2a[:, 0:CT2],
                                     start=True, stop=False)
                    tc.For_i_unrolled(
                        0, NBLK - 1, 1,
                        lambda iv: c2_body(iv, True, False, False,
                                           t_one1[:]),
                        max_unroll=4)
                    c2_body(NBLK - 1, False, False, True,
                            t_pm[:, NBLK - 1:NBLK])
                else:
                    for b in range(NBLK):
                        c2_body(b, False, b == 0, b == NBLK - 1,
                                t_pm[:, b:b + 1])
                t_po = constp.tile([1, CT2], f32)
                nc.vector.tensor_copy(t_po[:], t_pool[:])
                nc.sync.dma_start(d_pool[:], t_po[:])

    nc.compile()
    return nc


def _attr_array(att, CE, H, CH, scale):
    a = np.zeros((P, CE), ml_dtypes.bfloat16)
    CEH = CE // H
    att = np.asarray(att, np.float32).reshape(H, CH)
    for h in range(H):
        a[:, h * CEH:h * CEH + CH] = np.broadcast_to(
            (scale * att[h]).astype(ml_dtypes.bfloat16), (P, CH))
    return a


def _run(nc, maps):
    import time
    t0 = time.time()
    if os.environ.get("GAT_SIM", "0") == "1":
        from concourse import bass2jax
        r_results = bass2jax.run_bass_via_pjrt(nc, maps, n_cores=NC)

        class R:
            results = r_results
            exec_time_ns = None
        r = R()
    else:
        from concourse import bass_utils
        trace = bool(int(os.environ.get("GAT_TRACE", "0")))
        r = bass_utils.run_bass_kernel_spmd(nc, maps,
                                            core_ids=list(range(NC)),
                                            trace=trace)
    _cache.setdefault('run_wall', []).append(time.time() - t0)
    if getattr(r, 'exec_time_ns', None):
        _cache.setdefault('exec_ns', []).append(r.exec_time_ns)
    return r


def _warm_devices():
    # Establish the PJRT/axon connection (device discovery + first buffer)
    # while the host is busy with preprocessing and kernel build, so that
    # connection setup does not serialize with the actual launch.
    try:
        if os.environ.get("GAT_SIM", "0") == "1":
            return
        import jax
        d = jax.devices()
        jax.device_put(np.zeros((1,), np.float32), d[0]).block_until_ready()
    except Exception:
        pass


def kernel(x, edge_index, batch, Win, b_in, Wl1, bl1, Wr1, br1, att1, bias1,
           Wl2, bl2, Wr2, br2, att2, bias2, Wout, b_out):
    import threading
    warm = threading.Thread(target=_warm_devices, daemon=True)
    warm.start()
    x = np.asarray(x, np.float32)
    edge_index = np.asarray(edge_index)
    Win, b_in = np.asarray(Win, np.float32), np.asarray(b_in, np.float32)
    Wl1, bl1 = np.asarray(Wl1, np.float32), np.asarray(bl1, np.float32)
    Wr1, br1 = np.asarray(Wr1, np.float32), np.asarray(br1, np.float32)
    att1 = np.asarray(att1, np.float32)
    bias1 = np.asarray(bias1, np.float32)
    Wl2, bl2 = np.asarray(Wl2, np.float32), np.asarray(bl2, np.float32)
    Wr2, br2 = np.asarray(Wr2, np.float32), np.asarray(br2, np.float32)
    att2 = np.asarray(att2, np.float32)
    bias2 = np.asarray(bias2, np.float32)
    Wout, b_out = np.asarray(Wout, np.float32), np.asarray(b_out, np.float32)
    dbg = os.environ.get("GAT_DBG", "0") == "1"

    pre = _cache.get('pre')
    if pre is None or not np.array_equal(_cache.get('ei'), edge_index):
        pre = preprocess(edge_index)
        _cache['pre'] = pre
        _cache['ei'] = np.asarray(edge_index).copy()

    WA1, bA1 = Wl1 @ Win, Wl1 @ b_in + bl1
    WB1, bB1 = Wr1 @ Win, Wr1 @ b_in + br1
    flags = (bool(np.any(bA1)), bool(np.any(bB1)), bool(np.any(bias1)),
             bool(np.any(bl2)), bool(np.any(br2)), bool(np.any(bias2)))

    use_loop = (not dbg) and (not any(flags))
    key = ('nc', flags, dbg, use_loop)
    if key not in _cache:
        _cache[key] = build_fused(*flags, dbg=dbg, use_loop=use_loop)
    nc = _cache[key]

    bf = ml_dtypes.bfloat16
    at1a = _attr_array(att1, CE1, H1, CT1 // H1, 0.8)
    at1b = _attr_array(att1, CE1, H1, CT1 // H1, 0.2)
    at2a = _attr_array(att2, CE2, H2, CT2 // H2, 0.8)
    at2b = _attr_array(att2, CE2, H2, CT2 // H2, 0.2)
    WA1T = np.ascontiguousarray(WA1.T).astype(bf)
    WB1T = np.ascontiguousarray(WB1.T).astype(bf)
    WA2T = np.ascontiguousarray(Wl2.T).astype(bf)
    WB2T = np.ascontiguousarray(Wr2.T).astype(bf)

    maps = []
    for c in range(NC):
        xTc = np.zeros((P, PADN), bf)
        xTc[:, :NSH] = x[c * NSH:(c + 1) * NSH].T.astype(bf)
        m = {
            "xT": xTc,
            "idx": pre[c]['idx_all'],
            "slot": pre[c]['slot8'],
            "WA1": WA1T, "WB1": WB1T, "at1a": at1a, "at1b": at1b,
            "WA2": WA2T, "WB2": WB2T, "at2a": at2a, "at2b": at2b,
        }
        for nm, v, ct in (("b1l", bA1, CT1), ("b1r", bB1, CT1),
                          ("b1o", bias1, CT1), ("b2l", bl2, CT2),
                          ("b2r", br2, CT2), ("b2o", bias2, CT2)):
            if bool(np.any(v)):
                m[nm] = np.ascontiguousarray(
                    np.broadcast_to(v.astype(np.float32), (P, ct)))
        maps.append(m)

    warm.join(timeout=120)
    res = _run(nc, maps)
    if dbg:
        _cache['dbg'] = res.results
    pooled = sum(np.asarray(res.results[c]["pool_out"], np.float32)
                 for c in range(NC)).reshape(CT2)
    pooled = pooled / np.float32(N)
    out = pooled @ Wout.T + b_out
    return out[None, :].astype(np.float32)
